# revision 26
# baseline (speedup 1.0000x reference)
"""MoE layer (hash-routed, top-k=2, E=8 experts) on 8 Trainium2 NeuronCores.

Strategy: expert-parallel. Core e holds expert e's weights (W1[e], W2[e]).
The host routes: for each expert, gather the distinct tokens assigned to it
(assign[b,s,:] contains expert ids; a token contributes once per distinct
expert), transpose the gathered activations to [D, C] so the device never
has to transpose, run a dense 2-layer MLP per core, then scatter-add the
per-expert outputs back and divide by k.

All matmul operands are bf16 (same 1-cycle/row PE rate as f32r but half
the SBUF/DMA traffic and a fully-hidden LDWEIGHTS); accumulation stays in
fp32 PSUM and the SBUF y accumulator is fp32.  W1/W2/xt are pre-tiled on
the host into partition-major DRAM layouts so every DMA descriptor is a
2KB+ contiguous run.

Tokens are the moving/free dimension in BOTH layers (layer 2 keeps the
weight tile stationary and streams H1T, producing YT[d, tok]), so the PE
row count scales with the exact per-expert token capacity C = max expert
load -- no padding to a 128 multiple.

Device kernel (per core):
  for each quarter q of H (HQ columns):
    layer1: H1T[h, tok] = relu(W1q^T @ XT + b1q)   (PSUM-accum over d-tiles)
    layer2: YT[d, tok] += W2q^T @ H1T              (PSUM-accum over h-tiles,
                                                    SBUF f32 accum across q)
All loads ride one sync-HWDGE queue in consumption-priority order; 32
warm-up matmuls bridge the ~10us HBM fill phase so the PE clock ramp is
never reset by an idle gap.
"""

import math
import numpy as np

import concourse.bass as bass
import concourse.mybir as mybir
import concourse.tile as tile
from concourse import bacc
from concourse.bass_utils import run_bass_kernel_spmd

dt = mybir.dt

B, S, D, H, E, NCORES = 4, 1024, 1024, 4096, 8, 8
HQ = 1024                      # h-quarter width
KT = D // 128                  # 8 contraction tiles (d)
DT = D // 128                  # 8 output d-tiles
HTQ = HQ // 128                # 8 h-tiles per quarter
NQ = H // HQ                   # 4 quarters
HT = H // 128                  # 32 h-tiles total

# matmul operand dtype: "bf16" (half traffic, 1 cyc/row), "f32r" (4-byte
# full-rate path), "f32" (exact, 4x slower)
MM_DT = "bf16"

_BUILD_CACHE: dict = {}


def _io_np_dtype():
    if MM_DT == "bf16":
        import ml_dtypes

        return np.dtype(ml_dtypes.bfloat16)
    return np.dtype(np.float32)


def _io_dt():
    if MM_DT == "bf16":
        return dt.bfloat16
    if MM_DT == "f32r":
        return dt.float32r
    return dt.float32


def _chunks(C):
    """Token chunks of <=512 (PSUM bank limit for fp32 out)."""
    out = []
    c0 = 0
    while c0 < C:
        n = min(512, C - c0)
        out.append((c0, n))
        c0 += n
    return out


def build_nc(C: int):
    """Build + compile the per-core Bass program for token capacity C."""
    assert C % 4 == 0 and C >= 256
    assert NQ >= 2  # final-quarter y store lives in the add branch
    io_dt = _io_dt()

    nc = bacc.Bacc(
        "TRN2",
        target_bir_lowering=False,
        debug=False,
        num_devices=NCORES,
    )

    # pre-tiled partition-major layouts (built on the host):
    #   xt: [128, KT*C]      xt[p, kt*C+c]       = x_tok[c, kt*128+p]
    #   w1: [128, HT*KT*128] w1[p, ((ht*KT)+kt)*128+j] = W1[kt*128+p, ht*128+j]
    #   w2: [128, HT*D]      w2[p, hh*D+d]       = W2[hh*128+p, d]
    #   y:  [128, DT*C]      y[p, dt*C+c]        = out[tok c, dt*128+p]
    xt_d = nc.dram_tensor("xt", [128, KT * C], io_dt, kind="ExternalInput")
    w1_d = nc.dram_tensor("w1", [128, HT * KT * 128], io_dt, kind="ExternalInput")
    b1_d = nc.dram_tensor("b1", [H], dt.float32, kind="ExternalInput")
    w2_d = nc.dram_tensor("w2", [128, HT * D], io_dt, kind="ExternalInput")
    y_dt = dt.bfloat16 if MM_DT == "bf16" else dt.float32
    y_d = nc.dram_tensor("y", [128, DT * C], y_dt, kind="ExternalOutput")

    xt_v = xt_d.ap().rearrange("p (kt c) -> p kt c", kt=KT)
    w1_v = w1_d.ap().rearrange("p (ht kt h) -> p ht kt h", ht=HT, kt=KT)
    w2_v = w2_d.ap().rearrange("p (hh d) -> p hh d", hh=HT)
    b1_v = b1_d.ap().rearrange("(ht p) -> p ht", p=128)
    y_v = y_d.ap().rearrange("p (dt c) -> p dt c", dt=DT)

    esz = 2 if MM_DT == "bf16" else 4
    need = (
        KT * C * esz            # xt
        + DT * C * 4            # y accum (f32)
        + DT * C * (esz if MM_DT == "bf16" else 0)  # ybf
        + 2 * HTQ * KT * 128 * esz  # w1q double buffer
        + 2 * HTQ * 1024 * esz  # w2q double buffer
        + HTQ * C * esz         # h1q
        + 32 * 4                # b1
        + 1024                  # warm
    )
    assert need <= 190 * 1024, f"SBUF over budget: {need // 1024}KB for C={C}"

    n_chunks = _chunks(C)

    with tile.TileContext(nc) as tc:
        with (
            tc.tile_pool(name="xt", bufs=1) as xt_pool,
            tc.tile_pool(name="b1", bufs=1) as b1_pool,
            tc.tile_pool(name="y", bufs=1) as y_pool,
            tc.tile_pool(name="ybfp", bufs=1) as ybf_pool,
            tc.tile_pool(name="w1q", bufs=2) as w1_pool,
            tc.tile_pool(name="w2q", bufs=2) as w2_pool,
            tc.tile_pool(name="h1q", bufs=1) as h1_pool,
            tc.tile_pool(name="ps1", bufs=3, space="PSUM") as ps1_pool,
            tc.tile_pool(name="ps2", bufs=4, space="PSUM") as ps2_pool,
            tc.tile_pool(name="warm", bufs=1) as warm_pool,
            tc.tile_pool(name="warmps", bufs=1, space="PSUM") as warmps_pool,
        ):
            # PE warm-up: dependency-free bf16 matmuls bridge the ~10us
            # DMA fill phase so the HAM clock ramp isn't reset by an idle
            # gap right before the real stream
            wt = warm_pool.tile([128, 512], dt.bfloat16)
            nc.gpsimd.memset(wt[:], 0.0)
            wps = warmps_pool.tile([128, 512], dt.float32)
            for _ in range(38):
                nc.tensor.matmul(wps[:], wt[:, :128], wt[:], start=True, stop=True)

            # ALL loads go on the single sync HWDGE queue, issued in exact
            # consumption-priority order -- one ring's units fan out over
            # all 16 DMA engines, and strict in-queue ordering stops the
            # late-needed streams (w2q0, w1q1) from round-robin-stealing
            # bandwidth from the critical first ~15us (w1-ht0, xt).
            # y stores use the scalar HWDGE queue.
            b1t = b1_pool.tile([128, H // 128], dt.float32)
            nc.sync.dma_start(b1t[:], b1_v)

            w1q_cur = w1_pool.tile([128, HTQ, KT, 128], io_dt)
            nc.sync.dma_start(w1q_cur[:, 0:1], w1_v[:, 0:1])  # ht0 first

            xt = xt_pool.tile([128, KT, C], io_dt)
            for c0, n in n_chunks:
                nc.sync.dma_start(
                    xt[:, :, c0 : c0 + n], xt_v[:, :, c0 : c0 + n]
                )
            y = y_pool.tile([128, DT, C], dt.float32)
            ybf = None
            if y_dt == dt.bfloat16:
                ybf = ybf_pool.tile([128, DT, C], dt.bfloat16, name="ybf")

            for ht in range(1, HTQ):  # per-ht granularity for q0
                nc.sync.dma_start(
                    w1q_cur[:, ht : ht + 1], w1_v[:, ht : ht + 1]
                )

            def fetch_w1(q):
                """Whole-quarter W1 fetch (16KB per-partition descriptors);
                only quarter 0 needs per-ht arrival granularity."""
                w1q = w1_pool.tile([128, HTQ, KT, 128], io_dt)
                nc.sync.dma_start(
                    w1q[:], w1_v[:, q * HTQ : (q + 1) * HTQ]
                )
                return w1q

            def fetch_w2(q):
                w2q = w2_pool.tile([128, HTQ, 1024], io_dt)
                nc.sync.dma_start(
                    w2q[:], w2_v[:, q * HTQ : (q + 1) * HTQ]
                )
                return w2q

            w2q_cur = fetch_w2(0)
            w1q_nxt = fetch_w1(1)

            for q in range(NQ):
                if q >= 1:
                    w1q_cur, w1q_nxt = w1q_nxt, None
                    w2q_cur = fetch_w2(q)
                    if q + 1 < NQ:
                        w1q_nxt = fetch_w1(q + 1)
                w1q, w2q = w1q_cur, w2q_cur
                h1q = h1_pool.tile([128, HTQ, C], io_dt)

                # ---- layer 1: H1T[h, tok] = relu(W1q^T @ XT + b1) ----
                for ht in range(HTQ):
                    hcol = q * HQ + ht * 128
                    for c0, n in n_chunks:
                        ps = ps1_pool.tile([128, 512], dt.float32, tag="ps1")
                        for kt in range(KT):
                            nc.tensor.matmul(
                                ps[:, :n],
                                w1q[:, ht, kt],
                                xt[:, kt, c0 : c0 + n],
                                start=(kt == 0),
                                stop=(kt == KT - 1),
                            )
                        nc.scalar.activation(
                            h1q[:, ht, c0 : c0 + n],
                            ps[:, :n],
                            mybir.ActivationFunctionType.Relu,
                            bias=b1t[:, hcol // 128 : hcol // 128 + 1],
                        )

                # ---- layer 2: YT[d, tok] += W2q^T @ H1T ----
                for dtile in range(DT):
                    chunks2 = n_chunks
                    if q == NQ - 1 and dtile == DT - 1:
                        # split the very last chunk so its add+store
                        # overlaps the closing matmuls (shorter tail)
                        c0l, nl = n_chunks[-1]
                        h1 = (nl // 2 + 3) & ~3
                        chunks2 = n_chunks[:-1] + [
                            (c0l, h1),
                            (c0l + h1, nl - h1),
                        ]
                    for c0, n in chunks2:
                        ps = ps2_pool.tile([128, 512], dt.float32, tag="ps2")
                        for ht in range(HTQ):
                            nc.tensor.matmul(
                                ps[:, :n],
                                w2q[:, ht, dtile * 128 : (dtile + 1) * 128],
                                h1q[:, ht, c0 : c0 + n],
                                start=(ht == 0),
                                stop=(ht == HTQ - 1),
                            )
                        ys = y[:, dtile, c0 : c0 + n]
                        if q == 0:
                            nc.vector.tensor_copy(ys, ps[:, :n])
                        elif q < NQ - 1:
                            nc.vector.tensor_add(ys, ys, ps[:, :n])
                        else:
                            # final quarter: add straight into the bf16
                            # output tile (DVE converts on write), halving
                            # the store DMA on the tail
                            yo = ys if ybf is None else (
                                ybf[:, dtile, c0 : c0 + n]
                            )
                            nc.vector.tensor_add(yo, ys, ps[:, :n])
                            nc.scalar.dma_start(
                                y_v[:, dtile, c0 : c0 + n], yo
                            )

    nc.compile()
    return nc


def _get_nc(C: int):
    key = (C, MM_DT)
    if key not in _BUILD_CACHE:
        _BUILD_CACHE[key] = build_nc(C)
    return _BUILD_CACHE[key]


def _pretile_w1(w1e: np.ndarray, io_np) -> np.ndarray:
    # [D, H] -> [128, HT*KT*128]: w1[p, ((ht*KT)+kt)*128+j] = W1[kt*128+p, ht*128+j]
    return np.ascontiguousarray(
        w1e.reshape(KT, 128, HT, 128)
        .transpose(1, 2, 0, 3)
        .reshape(128, HT * KT * 128)
        .astype(io_np, copy=False)
    )


def _pretile_w2(w2e: np.ndarray, io_np) -> np.ndarray:
    # [H, D] -> [128, HT*D]: w2[p, hh*D+d] = W2[hh*128+p, d]
    return np.ascontiguousarray(
        w2e.reshape(HT, 128, D).transpose(1, 0, 2).reshape(128, HT * D)
        .astype(io_np, copy=False)
    )


def kernel(x, W1, b1, W2, b2, assign, k, _want_trace=False):
    x = np.asarray(x, dtype=np.float32)
    W1 = np.asarray(W1, dtype=np.float32)
    b1 = np.asarray(b1, dtype=np.float32)
    W2 = np.asarray(W2, dtype=np.float32)
    b2 = np.asarray(b2, dtype=np.float32)
    assign = np.asarray(assign)
    kk = int(k)

    assert W1.shape[0] == E and W2.shape[0] == E, "expert count must be 8"
    Bx, Sx, Dx = x.shape
    T = Bx * Sx
    xf = x.reshape(T, Dx)
    xT = np.ascontiguousarray(xf.T)  # [D, T]
    a2 = assign.reshape(T, -1)

    idx = [np.nonzero((a2 == e).any(axis=1))[0] for e in range(E)]
    max_n = max(len(i) for i in idx)

    # capacity per device pass = exact max expert load (tokens are the
    # free dim everywhere, so no 128-padding); multiple passes only if
    # pathologically skewed
    C = min(max(256, (max_n + 3) & ~3), 1280)
    n_pass = math.ceil(max(max_n, 1) / C)

    nc = _get_nc(C)
    io_np = _io_np_dtype()

    w1_io = [_pretile_w1(W1[e], io_np) for e in range(E)]
    w2_io = [_pretile_w2(W2[e], io_np) for e in range(E)]

    out_f = np.zeros((T, Dx), dtype=np.float32)
    trace_info = None

    for p in range(n_pass):
        in_maps = []
        for e in range(E):
            sl = idx[e][p * C : (p + 1) * C]
            # xt pre-tiled: [128, KT*C], xt[p, kt*C+c] = xT[kt*128+p, tok c]
            xt_buf = np.zeros((128, KT * C), dtype=io_np)
            if len(sl):
                xt_buf.reshape(128, KT, C)[:, :, : len(sl)] = (
                    xT[:, sl].reshape(KT, 128, len(sl)).transpose(1, 0, 2)
                    .astype(io_np, copy=False)
                )
            in_maps.append(
                {
                    "xt": xt_buf,
                    "w1": w1_io[e],
                    "b1": b1[e],
                    "w2": w2_io[e],
                }
            )
        res = run_bass_kernel_spmd(
            nc,
            in_maps,
            core_ids=list(range(NCORES)),
            trace=_want_trace,
            trace_cores=list(range(NCORES)) if _want_trace else None,
        )
        if _want_trace:
            trace_info = res
        for e in range(E):
            sl = idx[e][p * C : (p + 1) * C]
            if len(sl):
                # y comes back partition-major [128, DT*C] = YT[d, tok]
                yt = (
                    res.results[e]["y"]
                    .reshape(128, DT, C)
                    .transpose(1, 0, 2)
                    .reshape(Dx, C)[:, : len(sl)]
                )
                out_f[sl] += yt.T.astype(np.float32) + b2[e][None, :]

    out = (out_f * np.float32(1.0 / kk)).reshape(Bx, Sx, Dx)
    if _want_trace:
        return out, trace_info
    return out


# revision 30
# speedup vs baseline: 1.0103x; 1.0103x over previous
"""MoE layer (hash-routed, top-k=2, E=8 experts) on 8 Trainium2 NeuronCores.

Strategy: expert-parallel. Core e holds expert e's weights (W1[e], W2[e]).
The host routes: for each expert, gather the distinct tokens assigned to it
(assign[b,s,:] contains expert ids; a token contributes once per distinct
expert), transpose the gathered activations to [D, C] so the device never
has to transpose, run a dense 2-layer MLP per core, then scatter-add the
per-expert outputs back and divide by k.

All matmul operands are bf16 (same 1-cycle/row PE rate as f32r but half
the SBUF/DMA traffic and a fully-hidden LDWEIGHTS); accumulation stays in
fp32 PSUM and the SBUF y accumulator is fp32.  W1/W2/xt are pre-tiled on
the host into partition-major DRAM layouts so every DMA descriptor is a
2KB+ contiguous run.

Tokens are the moving/free dimension in BOTH layers (layer 2 keeps the
weight tile stationary and streams H1T, producing YT[d, tok]), so the PE
row count scales with the exact per-expert token capacity C = max expert
load -- no padding to a 128 multiple.

Device kernel (per core):
  for each quarter q of H (HQ columns):
    layer1: H1T[h, tok] = relu(W1q^T @ XT + b1q)   (PSUM-accum over d-tiles)
    layer2: YT[d, tok] += W2q^T @ H1T              (PSUM-accum over h-tiles,
                                                    SBUF f32 accum across q)
All loads ride one sync-HWDGE queue in consumption-priority order; 32
warm-up matmuls bridge the ~10us HBM fill phase so the PE clock ramp is
never reset by an idle gap.
"""

import math
import numpy as np

import concourse.bass as bass
import concourse.mybir as mybir
import concourse.tile as tile
from concourse import bacc
from concourse.bass_utils import run_bass_kernel_spmd

dt = mybir.dt

B, S, D, H, E, NCORES = 4, 1024, 1024, 4096, 8, 8
HQ = 1024                      # h-quarter width
KT = D // 128                  # 8 contraction tiles (d)
DT = D // 128                  # 8 output d-tiles
HTQ = HQ // 128                # 8 h-tiles per quarter
NQ = H // HQ                   # 4 quarters
HT = H // 128                  # 32 h-tiles total

# matmul operand dtype: "bf16" (half traffic, 1 cyc/row), "f32r" (4-byte
# full-rate path), "f32" (exact, 4x slower)
MM_DT = "bf16"

_BUILD_CACHE: dict = {}


def _io_np_dtype():
    if MM_DT == "bf16":
        import ml_dtypes

        return np.dtype(ml_dtypes.bfloat16)
    return np.dtype(np.float32)


def _io_dt():
    if MM_DT == "bf16":
        return dt.bfloat16
    if MM_DT == "f32r":
        return dt.float32r
    return dt.float32


def _chunks(C):
    """Token chunks of <=512 (PSUM bank limit for fp32 out)."""
    out = []
    c0 = 0
    while c0 < C:
        n = min(512, C - c0)
        out.append((c0, n))
        c0 += n
    return out


def build_nc(C: int):
    """Build + compile the per-core Bass program for token capacity C."""
    assert C % 4 == 0 and C >= 256
    assert NQ >= 2  # final-quarter y store lives in the add branch
    io_dt = _io_dt()

    nc = bacc.Bacc(
        "TRN2",
        target_bir_lowering=False,
        debug=False,
        num_devices=NCORES,
    )

    # pre-tiled partition-major layouts (built on the host):
    #   xt: [128, KT*C]      xt[p, kt*C+c]       = x_tok[c, kt*128+p]
    #   w1: [128, HT*KT*128] w1[p, ((ht*KT)+kt)*128+j] = W1[kt*128+p, ht*128+j]
    #   w2: [128, HT*D]      w2[p, hh*D+d]       = W2[hh*128+p, d]
    #   y:  [128, DT*C]      y[p, dt*C+c]        = out[tok c, dt*128+p]
    xt_d = nc.dram_tensor("xt", [128, KT * C], io_dt, kind="ExternalInput")
    w1_d = nc.dram_tensor("w1", [128, HT * KT * 128], io_dt, kind="ExternalInput")
    b1_d = nc.dram_tensor("b1", [H], dt.float32, kind="ExternalInput")
    w2_d = nc.dram_tensor("w2", [128, HT * D], io_dt, kind="ExternalInput")
    y_dt = dt.bfloat16 if MM_DT == "bf16" else dt.float32
    y_d = nc.dram_tensor("y", [128, DT * C], y_dt, kind="ExternalOutput")

    xt_v = xt_d.ap().rearrange("p (kt c) -> p kt c", kt=KT)
    w1_v = w1_d.ap().rearrange("p (ht kt h) -> p ht kt h", ht=HT, kt=KT)
    w2_v = w2_d.ap().rearrange("p (hh d) -> p hh d", hh=HT)
    b1_v = b1_d.ap().rearrange("(ht p) -> p ht", p=128)
    y_v = y_d.ap().rearrange("p (dt c) -> p dt c", dt=DT)

    esz = 2 if MM_DT == "bf16" else 4
    need = (
        KT * C * esz            # xt
        + DT * C * 4            # y accum (f32)
        + DT * C * (esz if MM_DT == "bf16" else 0)  # ybf
        + 2 * HTQ * KT * 128 * esz  # w1q double buffer
        + 2 * HTQ * 1024 * esz  # w2q double buffer
        + 2 * HTQ * C * esz     # h1q double buffer
        + 32 * 4                # b1
        + 1024                  # warm
    )
    assert need <= 190 * 1024, f"SBUF over budget: {need // 1024}KB for C={C}"

    n_chunks = _chunks(C)

    with tile.TileContext(nc) as tc:
        with (
            tc.tile_pool(name="xt", bufs=1) as xt_pool,
            tc.tile_pool(name="b1", bufs=1) as b1_pool,
            tc.tile_pool(name="y", bufs=1) as y_pool,
            tc.tile_pool(name="ybfp", bufs=1) as ybf_pool,
            tc.tile_pool(name="w1q", bufs=2) as w1_pool,
            tc.tile_pool(name="w2q", bufs=2) as w2_pool,
            tc.tile_pool(name="h1q", bufs=2) as h1_pool,
            tc.tile_pool(name="ps1", bufs=3, space="PSUM") as ps1_pool,
            tc.tile_pool(name="ps2", bufs=4, space="PSUM") as ps2_pool,
            tc.tile_pool(name="warm", bufs=1) as warm_pool,
            tc.tile_pool(name="warmps", bufs=1, space="PSUM") as warmps_pool,
        ):
            # PE warm-up: dependency-free bf16 matmuls bridge the ~10us
            # DMA fill phase so the HAM clock ramp isn't reset by an idle
            # gap right before the real stream
            wt = warm_pool.tile([128, 512], dt.bfloat16)
            nc.gpsimd.memset(wt[:], 0.0)
            wps = warmps_pool.tile([128, 512], dt.float32)
            for _ in range(34):
                nc.tensor.matmul(wps[:], wt[:, :128], wt[:], start=True, stop=True)

            # ALL loads go on the single sync HWDGE queue, issued in exact
            # consumption-priority order -- one ring's units fan out over
            # all 16 DMA engines, and strict in-queue ordering stops the
            # late-needed streams (w2q0, w1q1) from round-robin-stealing
            # bandwidth from the critical first ~15us (w1-ht0, xt).
            # y stores use the scalar HWDGE queue.
            w1q_cur = w1_pool.tile([128, HTQ, KT, 128], io_dt)
            nc.sync.dma_start(w1q_cur[:, 0:1], w1_v[:, 0:1])  # ht0 first

            xt = xt_pool.tile([128, KT, C], io_dt)
            for c0, n in n_chunks:
                nc.sync.dma_start(
                    xt[:, :, c0 : c0 + n], xt_v[:, :, c0 : c0 + n]
                )

            # b1 rides behind the critical prefix (first use ~19us in)
            b1t = b1_pool.tile([128, H // 128], dt.float32)
            nc.sync.dma_start(b1t[:], b1_v)
            y = y_pool.tile([128, DT, C], dt.float32)
            ybf = None
            if y_dt == dt.bfloat16:
                ybf = ybf_pool.tile([128, DT, C], dt.bfloat16, name="ybf")

            for ht in range(1, HTQ):  # per-ht granularity for q0
                nc.sync.dma_start(
                    w1q_cur[:, ht : ht + 1], w1_v[:, ht : ht + 1]
                )

            def fetch_w1(q):
                """Whole-quarter W1 fetch (16KB per-partition descriptors);
                only quarter 0 needs per-ht arrival granularity."""
                w1q = w1_pool.tile([128, HTQ, KT, 128], io_dt)
                nc.sync.dma_start(
                    w1q[:], w1_v[:, q * HTQ : (q + 1) * HTQ]
                )
                return w1q

            def fetch_w2(q):
                w2q = w2_pool.tile([128, HTQ, 1024], io_dt)
                nc.sync.dma_start(
                    w2q[:], w2_v[:, q * HTQ : (q + 1) * HTQ]
                )
                return w2q

            w2q_cur = fetch_w2(0)
            w1q_nxt = fetch_w1(1)

            for q in range(NQ):
                if q >= 1:
                    w1q_cur, w1q_nxt = w1q_nxt, None
                    w2q_cur = fetch_w2(q)
                    if q + 1 < NQ:
                        w1q_nxt = fetch_w1(q + 1)
                w1q, w2q = w1q_cur, w2q_cur
                h1q = h1_pool.tile([128, HTQ, C], io_dt)

                # ---- layer 1: H1T[h, tok] = relu(W1q^T @ XT + b1) ----
                for ht in range(HTQ):
                    hcol = q * HQ + ht * 128
                    for c0, n in n_chunks:
                        ps = ps1_pool.tile([128, 512], dt.float32, tag="ps1")
                        for kt in range(KT):
                            nc.tensor.matmul(
                                ps[:, :n],
                                w1q[:, ht, kt],
                                xt[:, kt, c0 : c0 + n],
                                start=(kt == 0),
                                stop=(kt == KT - 1),
                            )
                        nc.scalar.activation(
                            h1q[:, ht, c0 : c0 + n],
                            ps[:, :n],
                            mybir.ActivationFunctionType.Relu,
                            bias=b1t[:, hcol // 128 : hcol // 128 + 1],
                        )

                # ---- layer 2: YT[d, tok] += W2q^T @ H1T ----
                for dtile in range(DT):
                    chunks2 = n_chunks
                    if q == NQ - 1 and dtile == DT - 1:
                        # split the very last chunk so its add+store
                        # overlaps the closing matmuls (shorter tail)
                        c0l, nl = n_chunks[-1]
                        h1 = (nl // 2 + 3) & ~3
                        chunks2 = n_chunks[:-1] + [
                            (c0l, h1),
                            (c0l + h1, nl - h1),
                        ]
                    for c0, n in chunks2:
                        ps = ps2_pool.tile([128, 512], dt.float32, tag="ps2")
                        for ht in range(HTQ):
                            nc.tensor.matmul(
                                ps[:, :n],
                                w2q[:, ht, dtile * 128 : (dtile + 1) * 128],
                                h1q[:, ht, c0 : c0 + n],
                                start=(ht == 0),
                                stop=(ht == HTQ - 1),
                            )
                        ys = y[:, dtile, c0 : c0 + n]
                        if q == 0:
                            nc.vector.tensor_copy(ys, ps[:, :n])
                        elif q < NQ - 1:
                            nc.vector.tensor_add(ys, ys, ps[:, :n])
                        else:
                            # final quarter: add straight into the bf16
                            # output tile (DVE converts on write), halving
                            # the store DMA on the tail
                            yo = ys if ybf is None else (
                                ybf[:, dtile, c0 : c0 + n]
                            )
                            nc.vector.tensor_add(yo, ys, ps[:, :n])
                            nc.scalar.dma_start(
                                y_v[:, dtile, c0 : c0 + n], yo
                            )

    nc.compile()
    return nc


def _get_nc(C: int):
    key = (C, MM_DT)
    if key not in _BUILD_CACHE:
        _BUILD_CACHE[key] = build_nc(C)
    return _BUILD_CACHE[key]


def _pretile_w1(w1e: np.ndarray, io_np) -> np.ndarray:
    # [D, H] -> [128, HT*KT*128]: w1[p, ((ht*KT)+kt)*128+j] = W1[kt*128+p, ht*128+j]
    return np.ascontiguousarray(
        w1e.reshape(KT, 128, HT, 128)
        .transpose(1, 2, 0, 3)
        .reshape(128, HT * KT * 128)
        .astype(io_np, copy=False)
    )


def _pretile_w2(w2e: np.ndarray, io_np) -> np.ndarray:
    # [H, D] -> [128, HT*D]: w2[p, hh*D+d] = W2[hh*128+p, d]
    return np.ascontiguousarray(
        w2e.reshape(HT, 128, D).transpose(1, 0, 2).reshape(128, HT * D)
        .astype(io_np, copy=False)
    )


def kernel(x, W1, b1, W2, b2, assign, k, _want_trace=False):
    x = np.asarray(x, dtype=np.float32)
    W1 = np.asarray(W1, dtype=np.float32)
    b1 = np.asarray(b1, dtype=np.float32)
    W2 = np.asarray(W2, dtype=np.float32)
    b2 = np.asarray(b2, dtype=np.float32)
    assign = np.asarray(assign)
    kk = int(k)

    assert W1.shape[0] == E and W2.shape[0] == E, "expert count must be 8"
    Bx, Sx, Dx = x.shape
    T = Bx * Sx
    xf = x.reshape(T, Dx)
    xT = np.ascontiguousarray(xf.T)  # [D, T]
    a2 = assign.reshape(T, -1)

    idx = [np.nonzero((a2 == e).any(axis=1))[0] for e in range(E)]
    max_n = max(len(i) for i in idx)

    # capacity per device pass = exact max expert load (tokens are the
    # free dim everywhere, so no 128-padding); multiple passes only if
    # pathologically skewed
    C = min(max(256, (max_n + 3) & ~3), 1280)
    n_pass = math.ceil(max(max_n, 1) / C)

    nc = _get_nc(C)
    io_np = _io_np_dtype()

    w1_io = [_pretile_w1(W1[e], io_np) for e in range(E)]
    w2_io = [_pretile_w2(W2[e], io_np) for e in range(E)]

    out_f = np.zeros((T, Dx), dtype=np.float32)
    trace_info = None

    for p in range(n_pass):
        in_maps = []
        for e in range(E):
            sl = idx[e][p * C : (p + 1) * C]
            # xt pre-tiled: [128, KT*C], xt[p, kt*C+c] = xT[kt*128+p, tok c]
            xt_buf = np.zeros((128, KT * C), dtype=io_np)
            if len(sl):
                xt_buf.reshape(128, KT, C)[:, :, : len(sl)] = (
                    xT[:, sl].reshape(KT, 128, len(sl)).transpose(1, 0, 2)
                    .astype(io_np, copy=False)
                )
            in_maps.append(
                {
                    "xt": xt_buf,
                    "w1": w1_io[e],
                    "b1": b1[e],
                    "w2": w2_io[e],
                }
            )
        res = run_bass_kernel_spmd(
            nc,
            in_maps,
            core_ids=list(range(NCORES)),
            trace=_want_trace,
            trace_cores=list(range(NCORES)) if _want_trace else None,
        )
        if _want_trace:
            trace_info = res
        for e in range(E):
            sl = idx[e][p * C : (p + 1) * C]
            if len(sl):
                # y comes back partition-major [128, DT*C] = YT[d, tok]
                yt = (
                    res.results[e]["y"]
                    .reshape(128, DT, C)
                    .transpose(1, 0, 2)
                    .reshape(Dx, C)[:, : len(sl)]
                )
                out_f[sl] += yt.T.astype(np.float32) + b2[e][None, :]

    out = (out_f * np.float32(1.0 / kk)).reshape(Bx, Sx, Dx)
    if _want_trace:
        return out, trace_info
    return out
